# revision 1
# baseline (speedup 1.0000x reference)
"""Trainium2 Bass kernel for AffinityMatrixConstructLayer (v2, sharded).

Factorization: with G/H incidence matrices ([n,e], one-hot columns),
  M[(i2,i1),(k2,k1)] = sum_{j2,j1} G2[i2,j2]H2[k2,j2] Me[j2,j1] G1[i1,j1]H1[k1,j1]
                        + diag(Mp[i2,i1])
Per i2 block-row:
  S_T   = H2T * G2T[:,i2]              [192(j2), 48(k2)]
  C2T   = Me.T @ S_T                   [192(j1), 48(k2)]
  R     = C2T[:,k2,None] * H1T[:,None,k1]   [192(j1), 2304]
  rows  = G1T.T @ R                    [48(i1), 2304]

Sharding:
 - The d=1024 feature contraction for the affinity matrices is sharded
   128-per-core (each core holds a 128-row slice of Wn/We and 128-column
   slices of x1/x2/ef1/ef2); partial affinities are AllReduce-summed
   (153 KB) and everything downstream is computed per-core.
 - The 48 i2 block-rows are split 6-per-core. Each core's output columns
   are rotated by -6c blocks (via host-rotated edge tails) so the
   in-kernel diagonal add lands at a core-invariant position; the host
   un-rotates and concatenates. A host-passed one-hot `sel` [48,6]
   selects the core's Mp columns via a matmul.
"""

import sys

for _p in ("/opt/trn_rl_repo", "/root/.axon_site/_ro/trn_rl_repo"):
    if _p not in sys.path:
        sys.path.insert(0, _p)

import numpy as np

import concourse.bass as bass
import concourse.mybir as mybir
from concourse.tile import TileContext
from concourse.masks import make_identity
from concourse.bass_utils import run_bass_kernel_spmd

F32 = mybir.dt.float32
F32R = mybir.dt.float32r
I32 = mybir.dt.int32
AF = mybir.ActivationFunctionType
ALU = mybir.AluOpType

N_CORES = 8
N = 48          # nodes per graph
E = 192         # edges per graph
D = 1024        # feature dim
DS = D // N_CORES           # 128 feature dims per core
I2P = N // N_CORES          # 6 block-rows per core
ROWS = I2P * N              # 288 output rows per core
COLS = N * N                # 2304
AE_OFF = N * N
CC_LEN = N * N + E * E      # 39168

_CACHE: dict = {}
LAST_RESULTS = None


def _split_multiwaits(nc):
    """This walrus build encodes at most one sync-wait per instruction.
    Move extra waits onto injected single-wait drains on the same engine
    (engine queues execute in order, so semantics are preserved)."""
    for f in nc.m.functions:
        for blk in f.blocks:
            out = []
            for inst in blk.instructions:
                si = getattr(inst, "sync_info", None)
                if si is not None and si.on_wait and len(si.on_wait) > 1:
                    waits = list(si.on_wait)
                    for w in waits[:-1]:
                        d = mybir.InstDrain(
                            name=nc.get_next_instruction_name(),
                            ins=[], outs=[], bass_is_fusable=False)
                        d.engine = inst.engine
                        d.sync_info = mybir.SyncInfo(on_wait=[w], on_update=[])
                        out.append(d)
                    si.on_wait = waits[-1:]
                out.append(inst)
            try:
                blk.instructions[:] = out
            except TypeError:
                blk.instructions = out


def _softplus_relu(nc, spool, src_ap, out_ap, neghalf):
    """out = relu(softplus(src) - 0.5), stable:
    softplus(x) = relu(x) + ln(1 + exp(-|x|)); funcs all in one ACT set."""
    p, w = src_ap.shape[0], src_ap.shape[1]
    ab = spool.tile([p, w], F32, tag="sp_ab", name="sp_ab")
    nc.scalar.activation(ab, src_ap, AF.Abs)
    ex = spool.tile([p, w], F32, tag="sp_ex", name="sp_ex")
    nc.scalar.activation(ex, ab, AF.Exp, scale=-1.0)
    ln = spool.tile([p, w], F32, tag="sp_ln", name="sp_ln")
    nc.scalar.activation(ln, ex, AF.Ln, bias=1.0)
    rl = spool.tile([p, w], F32, tag="sp_rl", name="sp_rl")
    nc.scalar.activation(rl, src_ap, AF.Relu)
    pre = spool.tile([p, w], F32, tag="sp_pre", name="sp_pre")
    nc.vector.scalar_tensor_tensor(out=pre, in0=rl, scalar=-0.5, in1=ln,
                                   op0=ALU.add, op1=ALU.add)
    nc.scalar.activation(out_ap, pre, AF.Relu)


def _build() -> bass.Bass:
    if "nc" in _CACHE:
        return _CACHE["nc"]
    nc = bass.Bass(trn_type="TRN2", num_devices=N_CORES)

    d_Wn = nc.dram_tensor("Wn", [D, D], F32, kind="ExternalInput")
    d_We = nc.dram_tensor("We", [D, D], F32, kind="ExternalInput")
    d_gw = nc.dram_tensor("gw", [1, D], F32, kind="ExternalInput")
    d_bnbe = nc.dram_tensor("bnbe", [128, 16], F32, kind="ExternalInput")
    d_x1t = nc.dram_tensor("x1t", [D, N], F32, kind="ExternalInput")
    d_x2t = nc.dram_tensor("x2t", [D, N], F32, kind="ExternalInput")
    d_ef1t = nc.dram_tensor("ef1t", [D, E], F32, kind="ExternalInput")
    d_ef2t = nc.dram_tensor("ef2t", [D, E], F32, kind="ExternalInput")
    d_ei1 = nc.dram_tensor("ei1", [2, E], I32, kind="ExternalInput")
    d_ei2s = nc.dram_tensor("ei2s", [2, E], I32, kind="ExternalInput")
    d_sel = nc.dram_tensor("sel", [N, I2P], F32, kind="ExternalInput")
    d_out = nc.dram_tensor("out", [ROWS, COLS], F32, kind="ExternalOutput")
    d_mpr = nc.dram_tensor("mpr_scratch", [I2P, N], F32)

    KC = D // 128  # 8 contraction chunks

    with TileContext(nc) as tc:
        with (
            tc.tile_pool(name="const", bufs=1) as cpool,
            tc.tile_pool(name="wstream", bufs=4) as wpool,
            tc.tile_pool(name="scratch", bufs=2) as spool,
            tc.tile_pool(name="orow", bufs=3) as opool,
            tc.tile_pool(name="ptr", bufs=1, space="PSUM") as ptr,
            tc.tile_pool(name="paff", bufs=1, space="PSUM") as paff,
            tc.tile_pool(name="pout", bufs=2, space="PSUM") as pout,
            tc.tile_pool(name="pfin", bufs=2, space="PSUM") as pfin,
        ):
            # ---------- constants ----------
            ident = cpool.tile([128, 128], F32, tag="ident", name="ident")
            make_identity(nc, ident)
            id48 = cpool.tile([N, N], F32, tag="id48", name="id48")
            make_identity(nc, id48)
            iota48 = cpool.tile([128, N], F32, tag="iota48", name="iota48")
            nc.gpsimd.iota(iota48, pattern=[[1, N]], base=0,
                           channel_multiplier=0,
                           allow_small_or_imprecise_dtypes=True)
            iota6 = cpool.tile([128, I2P], F32, tag="iota6", name="iota6")
            nc.gpsimd.iota(iota6, pattern=[[1, I2P]], base=0,
                           channel_multiplier=0,
                           allow_small_or_imprecise_dtypes=True)
            neghalf = cpool.tile([128, 1], F32, tag="neghalf", name="neghalf")
            nc.vector.memset(neghalf, -0.5)

            # edge vectors -> f32 per-partition columns (cast on SWDGE DMA)
            ev_tiles = {}
            for tag, dt_ in (("e1", d_ei1), ("e2", d_ei2s)):
                for ci, (lo, hi) in enumerate(((0, 128), (128, 192))):
                    t = cpool.tile([hi - lo, 2], F32, tag=f"{tag}_{ci}",
                                   name=f"{tag}_{ci}")
                    nc.gpsimd.dma_start(
                        out=t, in_=dt_[:, lo:hi].rearrange("a b -> b a"))
                    ev_tiles[(tag, ci)] = t

            # incidence (transposed): X[j, node] = (edge_val[j] == node)
            def incid(tag_src, col, width, iota, tag, dt=F32):
                tiles = []
                for ci, p in ((0, 128), (1, 64)):
                    ev = ev_tiles[(tag_src, ci)][:, col:col + 1]
                    t = cpool.tile([p, width], dt, tag=f"{tag}{ci}",
                                   name=f"{tag}{ci}")
                    nc.vector.tensor_tensor(t, iota[0:p, :],
                                            ev.broadcast_to((p, width)),
                                            ALU.is_equal)
                    tiles.append(t)
                return tiles

            G1T = incid("e1", 0, N, iota48, "G1T", dt=F32R)
            H1T = incid("e1", 1, N, iota48, "H1T")
            G2T6 = incid("e2", 0, I2P, iota6, "G2T6")
            H2T = incid("e2", 1, N, iota48, "H2T")

            # ---------- loads + streaming matvec + affinities, ordered so
            # the edge-affinity (Me) critical path completes earliest ----
            gw_b = cpool.tile([128, D], F32, tag="gw_b", name="gw_b")
            nc.sync.dma_start(out=gw_b, in_=d_gw[:, :].broadcast_to((128, D)))
            bb_t = cpool.tile([128, 16], F32, tag="bb", name="bb")
            nc.sync.dma_start(out=bb_t, in_=d_bnbe[:, :])
            ef1T = cpool.tile([128, KC * E], F32, tag="ef1T", name="ef1T")
            ef2T = cpool.tile([128, KC * E], F32, tag="ef2T", name="ef2T")
            x1T = cpool.tile([128, KC * N], F32, tag="x1T", name="x1T")
            x2T = cpool.tile([128, KC * N], F32, tag="x2T", name="x2T")
            for dst, dsrc, w_ in ((ef1T, d_ef1t, E), (ef2T, d_ef2t, E)):
                nc.sync.dma_start(
                    out=dst.rearrange("p (k n) -> p k n", n=w_),
                    in_=dsrc[:, :].rearrange("(k p) n -> p k n", p=128))

            mv = cpool.tile([128, 16], F32, tag="mv", name="mv")
            coeff = cpool.tile([128, 16], F32, tag="coeff", name="coeff")
            a1T = cpool.tile([128, KC * N], F32, tag="a1T", name="a1T")
            aef1T = cpool.tile([128, KC * E], F32, tag="aef1T", name="aef1T")
            an_s = paff.tile([N, N], F32, tag="pa_n", name="an_s")
            ae_s = [paff.tile([128, E], F32, tag="pa_e", name="ae0"),
                    paff.tile([64, E], F32, tag="pa_e2", name="ae1")]

            def w_chunk(nm, dmat, k):
                wt = wpool.tile([128, D], F32, tag="w", name="wt")
                nc.sync.dma_start(out=wt, in_=dmat[k * 128:(k + 1) * 128, :])
                sc = spool.tile([128, D], F32, tag="sttout", name="sc")
                col = k if nm == "n" else KC + k
                cs = slice(col, col + 1)
                nc.vector.scalar_tensor_tensor(
                    out=sc, in0=wt, scalar=0.0, in1=gw_b,
                    op0=ALU.bypass, op1=ALU.mult, accum_out=mv[:, cs])
                # tanh(v) = 1 - 2/(exp(2v)+1); bb holds 2*b
                et = spool.tile([128, 1], F32, tag="et", name="et")
                nc.scalar.activation(et, mv[:, cs], AF.Exp, scale=2.0,
                                     bias=bb_t[:, cs])
                nc.vector.tensor_scalar_add(et, et, 1.0)
                rt = spool.tile([128, 1], F32, tag="rt", name="rt")
                nc.vector.reciprocal(rt, et)
                nc.vector.tensor_scalar(coeff[:, cs], rt, -2.0, 1.0,
                                        ALU.mult, ALU.add)
                if nm == "n":
                    ks = slice(k * N, (k + 1) * N)
                    nc.scalar.activation(a1T[:, ks], x1T[:, ks], AF.Copy,
                                         scale=coeff[:, cs])
                    nc.tensor.matmul(an_s, a1T[:, ks], x2T[:, ks],
                                     start=(k == 0), stop=(k == KC - 1))
                else:
                    ke = slice(k * E, (k + 1) * E)
                    nc.scalar.activation(aef1T[:, ke], ef1T[:, ke],
                                         AF.Copy, scale=coeff[:, cs])
                    for mi, (mlo, mhi) in enumerate(((0, 128), (128, 192))):
                        nc.tensor.matmul(
                            ae_s[mi], aef1T[:, k * E + mlo:k * E + mhi],
                            ef2T[:, ke],
                            start=(k == 0), stop=(k == KC - 1))

            for k in range(KC):
                w_chunk("e", d_We, k)
            for dst, dsrc, w_ in ((x1T, d_x1t, N), (x2T, d_x2t, N)):
                nc.sync.dma_start(
                    out=dst.rearrange("p (k n) -> p k n", n=w_),
                    in_=dsrc[:, :].rearrange("(k p) n -> p k n", p=128))
            for k in range(KC):
                w_chunk("n", d_Wn, k)
            # H1T tiled 48x along the free dim (constant across i2)
            h1tiled = []
            for ci, p in ((0, 128), (1, 64)):
                ht = cpool.tile([p, COLS], F32, tag=f"h1tl{ci}",
                                name=f"h1tl{ci}")
                nc.vector.tensor_copy(
                    ht.rearrange("p (a b) -> p a b", b=N),
                    H1T[ci].unsqueeze(1).broadcast_to((p, N, N)))
                h1tiled.append(ht)
            NT = [(t * 512, min(COLS, (t + 1) * 512))
                  for t in range((COLS + 511) // 512)]
            # B1[j1, i1*48+k1] = G1T[j1,i1] * H1T[j1,k1]   (f32r)
            b1 = []
            for ci, p in ((0, 128), (1, 64)):
                bt = cpool.tile([p, COLS], F32R, tag=f"b1{ci}",
                                name=f"b1{ci}")
                nc.vector.tensor_mul(
                    bt.rearrange("p (a b) -> p a b", b=N),
                    h1tiled[ci].rearrange("p (a b) -> p a b", b=N),
                    G1T[ci].unsqueeze(2).broadcast_to((p, N, N)))
                b1.append(bt)

            sel_sb = cpool.tile([N, I2P], F32, tag="sel_sb", name="sel_sb")
            nc.sync.dma_start(out=sel_sb, in_=d_sel[:, :])

            # ---------- nonlinearities ----------
            mp = cpool.tile([N, N], F32, tag="mp", name="mp")
            _softplus_relu(nc, spool, an_s, mp, neghalf)
            me = [cpool.tile([128, E], F32, tag="me_hi", name="me_hi"),
                  cpool.tile([64, E], F32, tag="me_lo", name="me_lo")]
            _softplus_relu(nc, spool, ae_s[0], me[0], neghalf)
            _softplus_relu(nc, spool, ae_s[1], me[1], neghalf)

            # Mp rows for owned i2s: [6, 48] via sel, bounced through DRAM
            # so each row can be placed on the partition (0 or 64) matching
            # the orow row it will be added to (DMA moves across partitions).
            pmp = ptr.tile([I2P, N], F32, tag="tr", name="pmp")
            nc.tensor.matmul(pmp, sel_sb, mp, start=True, stop=True)
            msel = spool.tile([I2P, N], F32, tag="msel", name="msel")
            nc.scalar.copy(msel, pmp)
            nc.sync.dma_start(out=d_mpr[:, :], in_=msel)
            mp_rows = []
            for i2 in range(I2P):
                off = 0 if i2 % 2 == 0 else 64
                mr = cpool.tile([off + 1, N], F32, tag=f"mpr{i2}",
                                name=f"mpr{i2}")
                nc.sync.dma_start(out=mr[off:off + 1, :],
                                  in_=d_mpr[i2:i2 + 1, :])
                mp_rows.append(mr[off:off + 1, :])

            # MeT via PE transposes: [192(j1), 192(j2)] in two row chunks, f32r
            ptm1 = ptr.tile([128, E], F32, tag="tr", name="ptm1")
            nc.tensor.transpose(ptm1[:, 0:128], me[0][:, 0:128], ident)
            nc.tensor.transpose(ptm1[:, 128:192], me[1][:, 0:128],
                                ident[0:64, 0:64])
            met_hi = cpool.tile([128, E], F32R, tag="met_hi", name="met_hi")
            nc.scalar.copy(met_hi, ptm1)
            ptm2 = ptr.tile([128, E], F32, tag="tr", name="ptm2")
            nc.tensor.transpose(ptm2[0:64, 0:128], me[0][:, 128:192], ident)
            nc.tensor.transpose(ptm2[0:64, 128:192], me[1][:, 128:192],
                                ident[0:64, 0:64])
            met_lo = cpool.tile([64, E], F32R, tag="met_lo", name="met_lo")
            nc.scalar.copy(met_lo, ptm2[0:64, :])

            # P = Me @ B1  [192(j2), 2304(i1,k1)], f32r, shared by all i2
            p_sb = [cpool.tile([128, COLS], F32R, tag="p_hi", name="p_hi"),
                    cpool.tile([64, COLS], F32R, tag="p_lo", name="p_lo")]
            for ms, (m0, m1) in enumerate(((0, 128), (128, 192))):
                for t0, t1 in NT:
                    pp = pout.tile([128, 512], F32, tag="po", name="pp")
                    w = t1 - t0
                    nc.tensor.matmul(pp[0:m1 - m0, 0:w], met_hi[:, m0:m1],
                                     b1[0][:, t0:t1], start=True, stop=False)
                    nc.tensor.matmul(pp[0:m1 - m0, 0:w], met_lo[:, m0:m1],
                                     b1[1][:, t0:t1], start=False, stop=True)
                    nc.scalar.copy(p_sb[ms][:, t0:t1], pp[0:m1 - m0, 0:w])

            for pa in range(I2P // 2):
                i2a, i2b = 2 * pa, 2 * pa + 1
                # lhsT [j2, 112]: cols 0:48 = S2(i2a), 64:112 = S2(i2b),
                # 48:64 zero. k2 axis of each S2 rotated by -i2 so the
                # diagonal row lands on partition 0 / 64 (host un-rotates).
                s2 = []
                for ci, p in ((0, 128), (1, 64)):
                    st = spool.tile([p, 112], F32R, tag=f"S{ci}",
                                    name=f"S{ci}")
                    nc.vector.memset(st[:, N:64].bitcast(F32), 0.0)
                    for off, i2 in ((0, i2a), (64, i2b)):
                        g2c = G2T6[ci][:, i2:i2 + 1]
                        nc.vector.tensor_scalar_mul(
                            st[:, off:off + N - i2], H2T[ci][:, i2:N], g2c)
                        if i2 > 0:
                            nc.vector.tensor_scalar_mul(
                                st[:, off + N - i2:off + N],
                                H2T[ci][:, 0:i2], g2c)
                    s2.append(st)
                orow = opool.tile([112, COLS], F32, tag="orow", name="orow")
                for t0, t1 in NT:
                    ps = pfin.tile([128, 512], F32, tag="pf", name="ps")
                    w = t1 - t0
                    nc.tensor.matmul(ps[0:112, 0:w], s2[0], p_sb[0][:, t0:t1],
                                     start=True, stop=False)
                    nc.tensor.matmul(ps[0:112, 0:w], s2[1], p_sb[1][:, t0:t1],
                                     start=False, stop=True)
                    eng = nc.vector if (t0 // 512) % 2 == 0 else nc.scalar
                    if eng is nc.vector:
                        nc.vector.tensor_copy(orow[:, t0:t1], ps[0:112, 0:w])
                    else:
                        nc.scalar.copy(orow[:, t0:t1], ps[0:112, 0:w])
                for off, i2 in ((0, i2a), (64, i2b)):
                    dg = orow[off:off + 1, 0:COLS:N + 1]
                    nc.vector.tensor_add(dg, dg, mp_rows[i2])
                    nc.sync.dma_start(out=d_out[i2 * N:(i2 + 1) * N, :],
                                      in_=orow[off:off + N, :])

    _split_multiwaits(nc)
    _CACHE["nc"] = nc
    return nc


def _make_in_maps(a):
    ei2 = a["edge_index2"].astype(np.int32)
    eye = np.eye(N, dtype=np.float32)
    bnbe = 2.0 * np.concatenate([
        a["bn"].reshape(8, 128).T, a["be"].reshape(8, 128).T,
    ], axis=1).astype(np.float32)  # [128, 16], col k = 2*(bn||be) chunk k
    x1t = np.ascontiguousarray(a["x1"].T)
    x2t = np.ascontiguousarray(a["x2"].T)
    ef1t = np.ascontiguousarray(a["ef1"].T)
    ef2t = np.ascontiguousarray(a["ef2"].T)
    in_maps = []
    for c in range(N_CORES):
        ei2s = np.stack([
            ei2[0] - I2P * c,                    # heads, shifted (match 0..5)
            (ei2[1] - I2P * c) % N,              # tails, rotated
        ]).astype(np.int32)
        in_maps.append({
            "Wn": a["Wn"],
            "We": a["We"],
            "gw": a["global_weight"].reshape(1, D),
            "bnbe": np.ascontiguousarray(bnbe),
            "x1t": x1t,
            "x2t": x2t,
            "ef1t": ef1t,
            "ef2t": ef2t,
            "ei1": a["edge_index1"].astype(np.int32),
            "ei2s": ei2s,
            "sel": np.ascontiguousarray(eye[:, I2P * c:I2P * (c + 1)]),
        })
    return in_maps


def kernel(**inputs) -> np.ndarray:
    global LAST_RESULTS
    nc = _build()
    a = {k: np.ascontiguousarray(np.asarray(v)) for k, v in inputs.items()}
    in_maps = _make_in_maps(a)
    res = run_bass_kernel_spmd(nc, in_maps, core_ids=list(range(N_CORES)))
    LAST_RESULTS = res

    parts = []
    for c in range(N_CORES):
        # device rows are [i2l, k2rot, (i1, k1)] with
        # k2g = (k2rot + i2l + 6c) mod 48; want [i2l, i1, (k2g, k1)]
        o = res.results[c]["out"].reshape(I2P, N, N, N).transpose(0, 2, 1, 3)
        o = np.stack([np.roll(o[i], i + I2P * c, axis=1)
                      for i in range(I2P)])
        parts.append(o.reshape(ROWS, COLS))
    return np.concatenate(parts, axis=0).astype(np.float32)


if __name__ == "__main__":
    _build()
    print("build OK")



# revision 15
# speedup vs baseline: 1.3789x; 1.3789x over previous
"""Trainium2 Bass kernel for AffinityMatrixConstructLayer (v3).

Math: M[(i2,i1),(k2,k1)] = sum_{j2,j1} G2[i2,j2]H2[k2,j2] Me[j2,j1]
                            G1[i1,j1]H1[k1,j1]  + diag(Mp)
where Me rows play the j2 role (e1==e2==192 makes the kron index
arithmetic alias me's ef1-row index to j2).

Key structure exploited: core c owns i2 block-rows [6c, 6c+6); only
graph-2 edges whose head lies in that range contribute, so the host
permutes graph-2 edges (and ef1 rows identically) to give each core a
compact owned slice of <= C=40 edges. Per core:
  - coeff = tanh(W@gw + b) via a bf16 streaming matvec on DVE
    (Wn/We host-cast to bf16: 4MB/core instead of 8MB)
  - one merged affinity GEMM: [ef1_own | x1_own] x [x2 | ef2]
    -> me [C,192] + mp [6,48] in a single psum accumulation
  - b1[j1, i1*48+k1] = (i1*48+k1 == hv1[j1]) via iota + is_equal
  - P = Me_own @ B1 [C, 2304]; orow = s2^T @ P per i2 pair, with the
    diagonal Mp add at a rotation-normalized position (host un-rotates)
PE is pre-warmed with junk matmuls on arriving W tiles so the GEMM
burst runs at 2.4 GHz.
"""

import sys

for _p in ("/opt/trn_rl_repo", "/root/.axon_site/_ro/trn_rl_repo"):
    if _p not in sys.path:
        sys.path.insert(0, _p)

import numpy as np
import ml_dtypes

import concourse.bass as bass
import concourse.mybir as mybir
from concourse.tile import TileContext
from concourse.masks import make_identity
from concourse.bass_utils import run_bass_kernel_spmd

F32 = mybir.dt.float32
F32R = mybir.dt.float32r
BF16 = mybir.dt.bfloat16
AF = mybir.ActivationFunctionType
ALU = mybir.AluOpType

N_CORES = 8
N = 48          # nodes per graph
E = 192         # edges per graph
D = 1024        # feature dim
I2P = N // N_CORES          # 6 block-rows per core
ROWS = I2P * N              # 288 output rows per core
COLS = N * N                # 2304
C = 40                      # padded owned-edge capacity per core
LW = C + 6                  # merged affinity lhs width (ef1_own | x1_own)
RW = N + E                  # merged affinity rhs width (x2 | ef2) = 240
KC = D // 128               # 8 contraction chunks

_CACHE: dict = {}
LAST_RESULTS = None


def _split_multiwaits(nc):
    """This walrus build encodes at most one sync-wait per instruction.
    Move extra waits onto injected single-wait drains on the same engine
    (engine queues execute in order, so semantics are preserved)."""
    for f in nc.m.functions:
        for blk in f.blocks:
            out = []
            for inst in blk.instructions:
                si = getattr(inst, "sync_info", None)
                if si is not None and si.on_wait and len(si.on_wait) > 1:
                    waits = list(si.on_wait)
                    for w in waits[:-1]:
                        d = mybir.InstDrain(
                            name=nc.get_next_instruction_name(),
                            ins=[], outs=[], bass_is_fusable=False)
                        d.engine = inst.engine
                        d.sync_info = mybir.SyncInfo(on_wait=[w], on_update=[])
                        out.append(d)
                    si.on_wait = waits[-1:]
                out.append(inst)
            try:
                blk.instructions[:] = out
            except TypeError:
                blk.instructions = out


def _build() -> bass.Bass:
    if "nc" in _CACHE:
        return _CACHE["nc"]
    nc = bass.Bass(trn_type="TRN2", num_devices=N_CORES)

    d_Wn = nc.dram_tensor("Wn", [D, D], BF16, kind="ExternalInput")
    d_We = nc.dram_tensor("We", [D, D], BF16, kind="ExternalInput")
    d_gw = nc.dram_tensor("gw", [1, D], BF16, kind="ExternalInput")
    d_bnbe = nc.dram_tensor("bnbe", [128, 16], F32, kind="ExternalInput")
    d_lhs = nc.dram_tensor("lhs", [D, LW], BF16, kind="ExternalInput")
    d_rhs = nc.dram_tensor("rhs", [D, RW], BF16, kind="ExternalInput")
    d_b1 = nc.dram_tensor("b1", [E, COLS], BF16, kind="ExternalInput")
    d_cv = nc.dram_tensor("cv", [C, 6], F32, kind="ExternalInput")
    d_out = nc.dram_tensor("out", [ROWS, COLS], F32, kind="ExternalOutput")
    d_mpr = nc.dram_tensor("mpr_scratch", [I2P, N], F32)

    with TileContext(nc) as tc:
        with (
            tc.tile_pool(name="const", bufs=1) as cpool,
            tc.tile_pool(name="wstream", bufs=4) as wpool,
            tc.tile_pool(name="scratch", bufs=2) as spool,
            tc.tile_pool(name="orow", bufs=3) as opool,
            tc.tile_pool(name="paff", bufs=1, space="PSUM") as paff,
            tc.tile_pool(name="pwarm", bufs=1, space="PSUM") as pwarm,
            tc.tile_pool(name="ptr", bufs=2, space="PSUM") as ptr,
            tc.tile_pool(name="pout", bufs=2, space="PSUM") as pout,
            tc.tile_pool(name="pfin", bufs=2, space="PSUM") as pfin,
        ):
            # ---------- small input DMAs (issue first; all on HWDGE) ----
            gw_b = cpool.tile([128, D], BF16, tag="gw_b", name="gw_b")
            nc.sync.dma_start(out=gw_b, in_=d_gw[:, :].broadcast_to((128, D)))
            bb_t = cpool.tile([128, 16], F32, tag="bb", name="bb")
            nc.sync.dma_start(out=bb_t, in_=d_bnbe[:, :])
            lhs = cpool.tile([128, KC * LW], BF16, tag="lhs", name="lhs")
            nc.sync.dma_start(
                out=lhs.rearrange("p (k n) -> p k n", n=LW),
                in_=d_lhs[:, :].rearrange("(k p) n -> p k n", p=128))
            rhs = cpool.tile([128, KC * RW], BF16, tag="rhs", name="rhs")
            nc.sync.dma_start(
                out=rhs.rearrange("p (k n) -> p k n", n=RW),
                in_=d_rhs[:, :].rearrange("(k p) n -> p k n", p=128))
            cv = cpool.tile([C, 6], F32, tag="cv", name="cv")
            nc.sync.dma_start(out=cv, in_=d_cv[:, :])

            # ---------- W stream (interleaved on two HWDGE queues) ------
            w_tiles = []
            for k in range(KC):
                wn = wpool.tile([128, D], BF16, tag="w", name=f"wn{k}")
                nc.sync.dma_start(out=wn, in_=d_Wn[k * 128:(k + 1) * 128, :])
                we = wpool.tile([128, D], BF16, tag="w", name=f"we{k}")
                nc.scalar.dma_start(out=we, in_=d_We[k * 128:(k + 1) * 128, :])
                w_tiles.append((wn, we))

            # b1 pattern is constant host data; lands in the idle DMA
            # window right after the W stream (per-queue FIFO keeps W first)
            b1_hi = cpool.tile([128, COLS], BF16, tag="b1_hi", name="b1_hi")
            nc.sync.dma_start(out=b1_hi, in_=d_b1[0:128, :])
            b1_lo = cpool.tile([64, COLS], BF16, tag="b1_lo", name="b1_lo")
            nc.scalar.dma_start(out=b1_lo, in_=d_b1[128:192, :])

            # ---------- constants / pattern tiles (built during stream) -
            ident = cpool.tile([128, 128], F32, tag="ident", name="ident")
            make_identity(nc, ident)
            iota112 = cpool.tile([C, 112], F32, tag="iota112", name="iota112")
            nc.gpsimd.iota(iota112, pattern=[[1, 112]], base=0,
                           channel_multiplier=0,
                           allow_small_or_imprecise_dtypes=True)

            # ACT table preload: dummy activation very early so the
            # natural_log_exp_and_others table load overlaps the W stream
            dum = spool.tile([1, 1], F32, tag="dum", name="dum")
            nc.vector.memset(dum, 0.0)
            nc.scalar.activation(dum, dum, AF.Exp)

            # s2 per pair: col (64*(i2%2) + k2rot) hot iff cv matches
            s2p = []
            for pa in range(3):
                sa = spool.tile([C, 112], F32, tag="s2a", name=f"s2a{pa}")
                nc.vector.tensor_tensor(
                    sa, iota112,
                    cv[:, 2 * pa:2 * pa + 1].broadcast_to((C, 112)),
                    ALU.is_equal)
                sb = spool.tile([C, 112], F32, tag="s2b", name=f"s2b{pa}")
                nc.vector.tensor_tensor(
                    sb, iota112,
                    cv[:, 2 * pa + 1:2 * pa + 2].broadcast_to((C, 112)),
                    ALU.is_equal)
                st = cpool.tile([C, 112], BF16, tag=f"s2{pa}", name=f"s2{pa}")
                nc.vector.tensor_tensor(st, sa, sb, ALU.add)
                s2p.append(st)

            # ---------- streaming matvec: mv[:, k] = (W chunk k) @ gw ---
            mv = cpool.tile([128, 16], F32, tag="mv", name="mv")
            scbf = spool.tile([128, D], BF16, tag="scbf", name="scbf")
            for k in range(KC):
                wn, we = w_tiles[k]
                nc.vector.scalar_tensor_tensor(
                    out=scbf, in0=wn, scalar=0.0, in1=gw_b,
                    op0=ALU.bypass, op1=ALU.mult, accum_out=mv[:, k:k + 1])
                nc.vector.scalar_tensor_tensor(
                    out=scbf, in0=we, scalar=0.0, in1=gw_b,
                    op0=ALU.bypass, op1=ALU.mult,
                    accum_out=mv[:, KC + k:KC + k + 1])

            # PE warmup on late W tiles (junk results, keeps HAM at 8/8
            # when the real GEMM burst arrives)
            warm = pwarm.tile([128, 512], F32, tag="warm", name="warm")
            for k in range(KC - 4, KC):
                for h in range(2):
                    nc.tensor.matmul(warm, w_tiles[k][h][:, 0:128],
                                     w_tiles[k][h][:, 0:512],
                                     start=True, stop=True)

            # ---------- tanh(mv + b) via exp (stays in one ACT set) -----
            mv2 = cpool.tile([128, 16], F32, tag="mv2", name="mv2")
            nc.vector.scalar_tensor_tensor(
                out=mv2, in0=mv, scalar=2.0, op0=ALU.mult,
                in1=bb_t, op1=ALU.add)
            et = spool.tile([128, 16], F32, tag="et", name="et")
            nc.scalar.activation(et, mv2, AF.Exp)
            nc.vector.tensor_scalar_add(et, et, 1.0)
            rt = spool.tile([128, 16], F32, tag="rt", name="rt")
            nc.vector.reciprocal(rt, et)
            coeff = cpool.tile([128, 16], F32, tag="coeff", name="coeff")
            nc.vector.tensor_scalar(coeff, rt, -2.0, 1.0, ALU.mult, ALU.add)

            # ---------- scaled lhs + merged affinity GEMM ---------------
            al = cpool.tile([128, KC * LW], BF16, tag="al", name="al")
            al3 = al.rearrange("p (k n) -> p k n", n=LW)
            lhs3 = lhs.rearrange("p (k n) -> p k n", n=LW)
            nc.vector.tensor_tensor(
                al3[:, :, 0:C], lhs3[:, :, 0:C],
                coeff[:, KC:16].unsqueeze(2).broadcast_to((128, KC, C)),
                ALU.mult)
            nc.vector.tensor_tensor(
                al3[:, :, C:LW], lhs3[:, :, C:LW],
                coeff[:, 0:KC].unsqueeze(2).broadcast_to((128, KC, I2P)),
                ALU.mult)
            aff = paff.tile([LW, RW], F32, tag="aff", name="aff")
            rhs3 = rhs.rearrange("p (k n) -> p k n", n=RW)
            for k in range(KC):
                nc.tensor.matmul(aff, al3[:, k, :], rhs3[:, k, :],
                                 start=(k == 0), stop=(k == KC - 1))

            # ---------- softplus_relu: relu(relu(x) + ln(e^-.5 + e^(-|x|-.5)))
            nh05 = cpool.tile([LW, 1], F32, tag="nh05", name="nh05")
            nc.vector.memset(nh05, -0.5)
            lnc = cpool.tile([LW, 1], F32, tag="lnc", name="lnc")
            nc.vector.memset(lnc, 0.6065306597126334)
            ab = spool.tile([LW, RW], F32, tag="sp_ab", name="sp_ab")
            nc.scalar.activation(ab, aff, AF.Abs)
            ex = spool.tile([LW, RW], F32, tag="sp_ex", name="sp_ex")
            nc.scalar.activation(ex, ab, AF.Exp, scale=-1.0, bias=nh05)
            ln = spool.tile([LW, RW], F32, tag="sp_ln", name="sp_ln")
            nc.scalar.activation(ln, ex, AF.Ln, bias=lnc)
            pre = spool.tile([LW, RW], F32, tag="sp_pre", name="sp_pre")
            nc.vector.scalar_tensor_tensor(
                out=pre, in0=aff, scalar=0.0, op0=ALU.max,
                in1=ln, op1=ALU.add)
            spall = cpool.tile([LW, RW], F32, tag="spall", name="spall")
            nc.scalar.activation(spall, pre, AF.Relu)

            # mp rows -> DRAM bounce to land on partitions 0/64
            nc.sync.dma_start(out=d_mpr[:, :], in_=spall[C:LW, 0:N])
            mp_rows = []
            for i2 in range(I2P):
                off = 0 if i2 % 2 == 0 else 64
                mr = cpool.tile([off + 1, N], F32, tag=f"mpr{i2}",
                                name=f"mpr{i2}")
                nc.sync.dma_start(out=mr[off:off + 1, :],
                                  in_=d_mpr[i2:i2 + 1, :])
                mp_rows.append(mr[off:off + 1, :])

            # MeT via PE transposes: [192(j1), C] in two row chunks, bf16
            ptm1 = ptr.tile([128, C], F32, tag="tr", name="ptm1")
            nc.tensor.transpose(ptm1, spall[0:C, N:N + 128], ident[0:C, 0:C])
            met_hi = cpool.tile([128, C], BF16, tag="met_hi", name="met_hi")
            nc.scalar.copy(met_hi, ptm1)
            ptm2 = ptr.tile([64, C], F32, tag="tr", name="ptm2")
            nc.tensor.transpose(ptm2, spall[0:C, N + 128:N + 192],
                                ident[0:C, 0:C])
            met_lo = cpool.tile([64, C], BF16, tag="met_lo", name="met_lo")
            nc.scalar.copy(met_lo, ptm2)

            # ---------- P = Me_own @ B1  [C, 2304] -> bf16 p_sb ---------
            NT = [(t * 512, min(COLS, (t + 1) * 512))
                  for t in range((COLS + 511) // 512)]
            p_sb = cpool.tile([C, COLS], BF16, tag="p_sb", name="p_sb")
            for t0, t1 in NT:
                w = t1 - t0
                pp = pout.tile([C, 512], F32, tag="po", name="pp")
                nc.tensor.matmul(pp[:, 0:w], met_hi, b1_hi[:, t0:t1],
                                 start=True, stop=False)
                nc.tensor.matmul(pp[:, 0:w], met_lo, b1_lo[:, t0:t1],
                                 start=False, stop=True)
                nc.vector.tensor_copy(p_sb[:, t0:t1], pp[:, 0:w])

            # ---------- finals: orow = s2^T @ P per pair + diag + out ---
            for pa in range(3):
                i2a, i2b = 2 * pa, 2 * pa + 1
                orow = opool.tile([112, COLS], F32, tag="orow", name="orow")
                for ti, (t0, t1) in enumerate(NT):
                    w = t1 - t0
                    ps = pfin.tile([128, 512], F32, tag="pf", name="ps")
                    nc.tensor.matmul(ps[0:112, 0:w], s2p[pa], p_sb[:, t0:t1],
                                     start=True, stop=True)
                    if ti % 2 == 0:
                        nc.vector.tensor_copy(orow[:, t0:t1], ps[0:112, 0:w])
                    else:
                        nc.scalar.copy(orow[:, t0:t1], ps[0:112, 0:w])
                for off, i2 in ((0, i2a), (64, i2b)):
                    dg = orow[off:off + 1, 0:COLS:N + 1]
                    nc.vector.tensor_add(dg, dg, mp_rows[i2])
                    nc.sync.dma_start(out=d_out[i2 * N:(i2 + 1) * N, :],
                                      in_=orow[off:off + N, :])

    _split_multiwaits(nc)
    _CACHE["nc"] = nc
    return nc


def _make_in_maps(a):
    bf = ml_dtypes.bfloat16
    ei1 = a["edge_index1"].astype(np.int64)
    ei2 = a["edge_index2"].astype(np.int64)
    heads2, tails2 = ei2[0], ei2[1]
    bnbe = 2.0 * np.concatenate([
        a["bn"].reshape(KC, 128).T, a["be"].reshape(KC, 128).T,
    ], axis=1).astype(np.float32)  # [128, 16], col k = 2*(bn||be) chunk k
    b1 = np.zeros((E, COLS), bf)
    b1[np.arange(E), ei1[0] * N + ei1[1]] = 1
    rhs = np.concatenate([a["x2"].T, a["ef2"].T], axis=1).astype(bf)
    wn = a["Wn"].astype(bf)
    we = a["We"].astype(bf)
    gw = a["global_weight"].reshape(1, D).astype(bf)

    in_maps = []
    owned_lists = []
    for c in range(N_CORES):
        owned = np.nonzero(heads2 // I2P == c)[0]
        assert len(owned) <= C, f"core {c} owns {len(owned)} > {C} edges"
        owned_lists.append(owned)
        # lhs = [ef1_owned | x1_owned] ^T, bf16  [D, C+6]
        ef1o = np.zeros((C, D), np.float32)
        ef1o[:len(owned)] = a["ef1"][owned]
        lhs = np.concatenate(
            [ef1o.T, a["x1"][I2P * c:I2P * (c + 1)].T], axis=1).astype(bf)
        # cv[s, i2] = rotated tail + 64*(i2%2) if head matches else 999
        cvm = np.full((C, 6), 999.0, np.float32)
        for s, j2 in enumerate(owned):
            hl = heads2[j2] - I2P * c
            cvm[s, hl] = (tails2[j2] - I2P * c - hl) % N + 64 * (hl % 2)
        in_maps.append({
            "Wn": wn, "We": we, "gw": gw,
            "bnbe": np.ascontiguousarray(bnbe),
            "lhs": np.ascontiguousarray(lhs),
            "rhs": np.ascontiguousarray(rhs),
            "b1": b1, "cv": cvm,
        })
    return in_maps


def kernel(**inputs) -> np.ndarray:
    global LAST_RESULTS
    nc = _build()
    a = {k: np.ascontiguousarray(np.asarray(v)) for k, v in inputs.items()}
    in_maps = _make_in_maps(a)
    res = run_bass_kernel_spmd(nc, in_maps, core_ids=list(range(N_CORES)))
    LAST_RESULTS = res

    parts = []
    for c in range(N_CORES):
        # device rows are [i2l, k2rot, (i1, k1)] with
        # k2g = (k2rot + i2l + 6c) mod 48; want [i2l, i1, (k2g, k1)]
        o = res.results[c]["out"].reshape(I2P, N, N, N).transpose(0, 2, 1, 3)
        o = np.stack([np.roll(o[i], i + I2P * c, axis=1)
                      for i in range(I2P)])
        parts.append(o.reshape(ROWS, COLS))
    return np.concatenate(parts, axis=0).astype(np.float32)


if __name__ == "__main__":
    _build()
    print("build OK")


# revision 31
# speedup vs baseline: 1.4052x; 1.0190x over previous
"""Trainium2 Bass kernel for AffinityMatrixConstructLayer (v4).

Math: M[(i2,i1),(k2,k1)] = sum_{j2,j1} G2[i2,j2]H2[k2,j2] Me[j2,j1]
                            G1[i1,j1]H1[k1,j1]  + diag(Mp)
where Me rows play the j2 role (e1==e2==192 makes the kron index
arithmetic alias me's ef1-row index to j2).

Structure exploited per core c (owns i2 block-rows [6c, 6c+6)):
  - only graph-2 edges with head in range contribute; host permutes
    edges + ef1 rows so the owned slice is compact (C=40 padded)
  - coeff = tanh(W@gw+b): bf16 W stream (4MB/core) + fused
    tensor_tensor_reduce matvec on DVE (2x bf16 mode)
  - one merged affinity GEMM [ef1_own | x1_own] x [x2 | ef2]
  - P = Me_own @ B1 (b1 one-hot pattern is host-built, DMA'd bf16)
  - diag(Mp) folded into the final GEMM: p_sb rows C..C+5 carry
    strided mp diagonals, cv rows C..C+5 make s2 route them to the
    k2rot=0 output rows. No DRAM bounce, no separate diag add.
  - PE pre-warmed with junk matmuls on arriving W tiles.
"""

import sys

for _p in ("/opt/trn_rl_repo", "/root/.axon_site/_ro/trn_rl_repo"):
    if _p not in sys.path:
        sys.path.insert(0, _p)

import numpy as np
import ml_dtypes

import concourse.bass as bass
import concourse.mybir as mybir
from concourse.tile import TileContext
from concourse.masks import make_identity
from concourse.bass_utils import run_bass_kernel_spmd

F32 = mybir.dt.float32
BF16 = mybir.dt.bfloat16
AF = mybir.ActivationFunctionType
ALU = mybir.AluOpType

N_CORES = 8
N = 48          # nodes per graph
E = 192         # edges per graph
D = 1024        # feature dim
I2P = N // N_CORES          # 6 block-rows per core
ROWS = I2P * N              # 288 output rows per core
COLS = N * N                # 2304
C = 32                      # padded owned-edge capacity per core
CD = C + 6                  # + 6 mp-diagonal delta rows
LW = C + 6                  # merged affinity lhs width (ef1_own | x1_own)
RW = N + E                  # merged affinity rhs width (x2 | ef2) = 240
KC = D // 128               # 8 contraction chunks

_CACHE: dict = {}
LAST_RESULTS = None


def _split_multiwaits(nc):
    """This walrus build encodes at most one sync-wait per instruction.
    Move extra waits onto injected single-wait drains on the same engine
    (engine queues execute in order, so semantics are preserved)."""
    for f in nc.m.functions:
        for blk in f.blocks:
            out = []
            for inst in blk.instructions:
                si = getattr(inst, "sync_info", None)
                if si is not None and si.on_wait and len(si.on_wait) > 1:
                    waits = list(si.on_wait)
                    for w in waits[:-1]:
                        d = mybir.InstDrain(
                            name=nc.get_next_instruction_name(),
                            ins=[], outs=[], bass_is_fusable=False)
                        d.engine = inst.engine
                        d.sync_info = mybir.SyncInfo(on_wait=[w], on_update=[])
                        out.append(d)
                    si.on_wait = waits[-1:]
                out.append(inst)
            try:
                blk.instructions[:] = out
            except TypeError:
                blk.instructions = out


def _build() -> bass.Bass:
    if "nc" in _CACHE:
        return _CACHE["nc"]
    nc = bass.Bass(trn_type="TRN2", num_devices=N_CORES)

    # Wn/We are host-TRANSPOSED: [din, dout] so the PE matvec can use gw
    # as the stationary operand (1-col LDWEIGHTS) and W as the moving one
    d_Wn = nc.dram_tensor("Wn", [D, D], BF16, kind="ExternalInput")
    d_We = nc.dram_tensor("We", [D, D], BF16, kind="ExternalInput")
    d_gw = nc.dram_tensor("gw", [128, KC], BF16, kind="ExternalInput")
    d_mvs = nc.dram_tensor("mv_scratch", [4, 512], F32)
    d_bnbe = nc.dram_tensor("bnbe", [128, 16], F32, kind="ExternalInput")
    d_lhs = nc.dram_tensor("lhs", [128, KC * LW], BF16, kind="ExternalInput")
    d_rhs = nc.dram_tensor("rhs", [128, KC * RW], BF16, kind="ExternalInput")
    d_b1 = nc.dram_tensor("b1", [E, COLS], BF16, kind="ExternalInput")
    d_cv = nc.dram_tensor("cv", [CD, 6], F32, kind="ExternalInput")
    d_out = nc.dram_tensor("out", [ROWS, COLS], F32, kind="ExternalOutput")

    with TileContext(nc) as tc:
        with (
            tc.tile_pool(name="const", bufs=1) as cpool,
            tc.tile_pool(name="wstream", bufs=4) as wpool,
            tc.tile_pool(name="scratch", bufs=2) as spool,
            tc.tile_pool(name="orow", bufs=3) as opool,
            tc.tile_pool(name="paff", bufs=1, space="PSUM") as paff,
            tc.tile_pool(name="pmv", bufs=1, space="PSUM") as pmv,
            tc.tile_pool(name="pout", bufs=2, space="PSUM") as pout,
            tc.tile_pool(name="pfin", bufs=2, space="PSUM") as pfin,
        ):
            # ---- queue sync: bb, Wn stream, rhs, b1_hi, (outputs later)
            bb_t = cpool.tile([128, 16], F32, tag="bb", name="bb")
            nc.sync.dma_start(out=bb_t, in_=d_bnbe[:, :])
            # ---- queue scalar: cv, gw, We stream, lhs, b1_lo
            cv = cpool.tile([CD, 6], F32, tag="cv", name="cv")
            nc.scalar.dma_start(out=cv, in_=d_cv[:, :])
            gwp = cpool.tile([128, KC], BF16, tag="gwp", name="gwp")
            nc.scalar.dma_start(out=gwp, in_=d_gw[:, :])

            w_tiles = []
            for k in range(KC):
                wn = wpool.tile([128, D], BF16, tag="w", name=f"wn{k}")
                nc.sync.dma_start(out=wn, in_=d_Wn[k * 128:(k + 1) * 128, :])
                we = wpool.tile([128, D], BF16, tag="w", name=f"we{k}")
                nc.scalar.dma_start(out=we, in_=d_We[k * 128:(k + 1) * 128, :])
                w_tiles.append((wn, we))

            rhs = cpool.tile([128, KC * RW], BF16, tag="rhs", name="rhs")
            nc.sync.dma_start(out=rhs, in_=d_rhs[:, :])
            b1_hi = cpool.tile([128, COLS], BF16, tag="b1_hi", name="b1_hi")
            nc.sync.dma_start(out=b1_hi, in_=d_b1[0:128, :])
            lhs = cpool.tile([128, KC * LW], BF16, tag="lhs", name="lhs")
            nc.scalar.dma_start(out=lhs, in_=d_lhs[:, :])
            b1_lo = cpool.tile([64, COLS], BF16, tag="b1_lo", name="b1_lo")
            nc.scalar.dma_start(out=b1_lo, in_=d_b1[128:192, :])

            # ---------- constants (built during stream) -----------------
            ident = cpool.tile([128, 128], F32, tag="ident", name="ident")
            make_identity(nc, ident)
            iota112 = cpool.tile([CD, 112], F32, tag="iota112", name="i112")
            nc.gpsimd.iota(iota112, pattern=[[1, 112]], base=0,
                           channel_multiplier=0,
                           allow_small_or_imprecise_dtypes=True)

            # ACT table preload overlaps the W stream
            dum = spool.tile([1, 1], F32, tag="dum", name="dum")
            nc.vector.memset(dum, 0.0)
            nc.scalar.activation(dum, dum, AF.Exp)

            # p_sb rows C..C+5 (mp diag deltas): zero the background once
            # (tile padded to 64 partitions for gpsimd's 32-alignment rule)
            p_sb = cpool.tile([64, COLS], BF16, tag="p_sb", name="p_sb")
            nc.gpsimd.memset(p_sb, 0.0)

            # s2 per pair: col (64*(i2%2) + k2rot) hot iff cv matches;
            # rows C..C+5 route the mp-diag delta rows of p_sb
            s2p = []
            for pa in range(3):
                sa = spool.tile([CD, 112], F32, tag="s2a", name=f"s2a{pa}")
                nc.vector.tensor_tensor(
                    sa, iota112,
                    cv[:, 2 * pa:2 * pa + 1].broadcast_to((CD, 112)),
                    ALU.is_equal)
                sb = spool.tile([CD, 112], F32, tag="s2b", name=f"s2b{pa}")
                nc.vector.tensor_tensor(
                    sb, iota112,
                    cv[:, 2 * pa + 1:2 * pa + 2].broadcast_to((CD, 112)),
                    ALU.is_equal)
                st = cpool.tile([CD, 112], BF16, tag=f"s2{pa}", name=f"s2{pa}")
                nc.vector.tensor_tensor(st, sa, sb, ALU.add)
                s2p.append(st)

            # ---------- streaming PE matvec: gw stationary (1-col weights)
            # pmv_a rows {0,32} = Wn@gw halves, pmv_b = We@gw halves.
            # These matmuls also keep the PE HAM-warm through the stream.
            pmva = pmv.tile([33, 512], F32, tag="pmva", name="pmva")
            pmvb = pmv.tile([33, 512], F32, tag="pmvb", name="pmvb")
            for k in range(KC):
                wn, we = w_tiles[k]
                for h in range(2):
                    nc.tensor.matmul(pmva[32 * h:32 * h + 1, :],
                                     gwp[:, k:k + 1],
                                     wn[:, 512 * h:512 * h + 512],
                                     start=(k == 0), stop=(k == KC - 1))
                    nc.tensor.matmul(pmvb[32 * h:32 * h + 1, :],
                                     gwp[:, k:k + 1],
                                     we[:, 512 * h:512 * h + 512],
                                     start=(k == 0), stop=(k == KC - 1))

            # gather [1,512] rows into per-partition [128, 16] layout:
            # psum -> sbuf copies, then 4 partition-redistributing DMAs
            mva = spool.tile([33, 512], F32, tag="mva", name="mva")
            nc.vector.tensor_copy(mva, pmva)
            mvb = spool.tile([33, 512], F32, tag="mvb", name="mvb")
            nc.vector.tensor_copy(mvb, pmvb)
            nc.sync.dma_start(out=d_mvs[0:2, :], in_=mva[0:33:32, :])
            nc.scalar.dma_start(out=d_mvs[2:4, :], in_=mvb[0:33:32, :])
            mv = cpool.tile([128, 16], F32, tag="mv", name="mv")
            nc.sync.dma_start(
                out=mv,
                in_=d_mvs[:, :].rearrange("r (kc p) -> p (r kc)", p=128))

            # PE keep-warm bridge while the gather DMAs run
            wrm = pfin.tile([128, 512], F32, tag="pf", name="wrm")
            nc.tensor.matmul(wrm, rhs[:, 0:128], rhs[:, 0:512],
                             start=True, stop=True)
            nc.tensor.matmul(wrm, rhs[:, 0:128], rhs[:, 512:1024],
                             start=True, stop=True)

            # ---------- tanh(mv + b) via exp (stays in one ACT set) -----
            mv2 = cpool.tile([128, 16], F32, tag="mv2", name="mv2")
            nc.vector.scalar_tensor_tensor(
                out=mv2, in0=mv, scalar=2.0, op0=ALU.mult,
                in1=bb_t, op1=ALU.add)
            et = spool.tile([128, 16], F32, tag="et", name="et")
            nc.scalar.activation(et, mv2, AF.Exp)
            nc.vector.tensor_scalar_add(et, et, 1.0)
            rt = spool.tile([128, 16], F32, tag="rt", name="rt")
            nc.vector.reciprocal(rt, et)
            coeff = cpool.tile([128, 16], F32, tag="coeff", name="coeff")
            nc.vector.tensor_scalar(coeff, rt, -2.0, 1.0, ALU.mult, ALU.add)

            # ---------- scaled lhs + merged affinity GEMM ---------------
            al = cpool.tile([128, KC * LW], BF16, tag="al", name="al")
            al3 = al.rearrange("p (k n) -> p k n", n=LW)
            lhs3 = lhs.rearrange("p (k n) -> p k n", n=LW)
            nc.vector.tensor_tensor(
                al3[:, :, 0:C], lhs3[:, :, 0:C],
                coeff[:, KC:16].unsqueeze(2).broadcast_to((128, KC, C)),
                ALU.mult)
            nc.vector.tensor_tensor(
                al3[:, :, C:LW], lhs3[:, :, C:LW],
                coeff[:, 0:KC].unsqueeze(2).broadcast_to((128, KC, I2P)),
                ALU.mult)
            aff = paff.tile([LW, RW], F32, tag="aff", name="aff")
            rhs3 = rhs.rearrange("p (k n) -> p k n", n=RW)
            for k in range(KC):
                nc.tensor.matmul(aff, al3[:, k, :], rhs3[:, k, :],
                                 start=(k == 0), stop=(k == KC - 1))

            # ---------- softplus_relu: relu(relu(x) + ln(e^-.5 + e^(-|x|-.5)))
            nh05 = cpool.tile([LW, 1], F32, tag="nh05", name="nh05")
            nc.vector.memset(nh05, -0.5)
            lnc = cpool.tile([LW, 1], F32, tag="lnc", name="lnc")
            nc.vector.memset(lnc, 0.6065306597126334)
            ab = spool.tile([LW, RW], F32, tag="sp_ab", name="sp_ab")
            nc.scalar.activation(ab, aff, AF.Abs)
            ex = spool.tile([LW, RW], F32, tag="sp_ex", name="sp_ex")
            nc.scalar.activation(ex, ab, AF.Exp, scale=-1.0, bias=nh05)
            ln = spool.tile([LW, RW], F32, tag="sp_ln", name="sp_ln")
            nc.scalar.activation(ln, ex, AF.Ln, bias=lnc)
            pre = spool.tile([LW, RW], F32, tag="sp_pre", name="sp_pre")
            nc.vector.scalar_tensor_tensor(
                out=pre, in0=aff, scalar=0.0, op0=ALU.max,
                in1=ln, op1=ALU.add)
            spall = cpool.tile([LW, RW], F32, tag="spall", name="spall")
            nc.scalar.activation(spall, pre, AF.Relu)

            # mp diag deltas: one strided copy, partitions C..C+5 (base 32)
            nc.vector.tensor_copy(p_sb[C:CD, 0:COLS:N + 1],
                                  spall[C:CD, 0:N])

            # MeT via PE transposes: [192(j1), C] in two row chunks, bf16
            ptm1 = pout.tile([128, C], F32, tag="po", name="ptm1")
            nc.tensor.transpose(ptm1, spall[0:C, N:N + 128], ident[0:C, 0:C])
            met_hi = cpool.tile([128, C], BF16, tag="met_hi", name="met_hi")
            nc.scalar.copy(met_hi, ptm1)
            ptm2 = pout.tile([64, C], F32, tag="po", name="ptm2")
            nc.tensor.transpose(ptm2, spall[0:C, N + 128:N + 192],
                                ident[0:C, 0:C])
            met_lo = cpool.tile([64, C], BF16, tag="met_lo", name="met_lo")
            nc.scalar.copy(met_lo, ptm2)

            # ---------- P = Me_own @ B1  [C, 2304] -> bf16 p_sb ---------
            NT = [(t * 512, min(COLS, (t + 1) * 512))
                  for t in range((COLS + 511) // 512)]
            for t0, t1 in NT:
                w = t1 - t0
                pp = pout.tile([C, 512], F32, tag="po", name="pp")
                nc.tensor.matmul(pp[:, 0:w], met_hi, b1_hi[:, t0:t1],
                                 start=True, stop=False)
                nc.tensor.matmul(pp[:, 0:w], met_lo, b1_lo[:, t0:t1],
                                 start=False, stop=True)
                nc.vector.tensor_copy(p_sb[0:C, t0:t1], pp[:, 0:w])

            # ---------- finals: orow = s2^T @ p_sb per pair + out DMA ---
            for pa in range(3):
                i2a, i2b = 2 * pa, 2 * pa + 1
                orow = opool.tile([112, COLS], F32, tag="orow", name="orow")
                for ti, (t0, t1) in enumerate(NT):
                    w = t1 - t0
                    ps = pfin.tile([128, 512], F32, tag="pf", name="ps")
                    nc.tensor.matmul(ps[0:112, 0:w], s2p[pa],
                                     p_sb[0:CD, t0:t1],
                                     start=True, stop=True)
                    if ti % 2 == 0:
                        nc.vector.tensor_copy(orow[:, t0:t1], ps[0:112, 0:w])
                    else:
                        nc.scalar.copy(orow[:, t0:t1], ps[0:112, 0:w])
                for off, i2 in ((0, i2a), (64, i2b)):
                    nc.sync.dma_start(out=d_out[i2 * N:(i2 + 1) * N, :],
                                      in_=orow[off:off + N, :])

    _split_multiwaits(nc)
    _CACHE["nc"] = nc
    return nc


def _make_in_maps(a):
    bf = ml_dtypes.bfloat16
    ei1 = a["edge_index1"].astype(np.int64)
    ei2 = a["edge_index2"].astype(np.int64)
    heads2, tails2 = ei2[0], ei2[1]
    bnbe = 2.0 * np.concatenate([
        a["bn"].reshape(KC, 128).T, a["be"].reshape(KC, 128).T,
    ], axis=1).astype(np.float32)  # [128, 16], col k = 2*(bn||be) chunk k
    b1 = np.zeros((E, COLS), bf)
    b1[np.arange(E), ei1[0] * N + ei1[1]] = 1
    # rhs [x2^T | ef2^T] pre-permuted to [128, KC*RW] (p-major chunks)
    rhs_f = np.concatenate([a["x2"].T, a["ef2"].T], axis=1)  # [D, RW]
    rhs = np.ascontiguousarray(
        rhs_f.reshape(KC, 128, RW).transpose(1, 0, 2).reshape(128, KC * RW)
    ).astype(bf)
    gw = np.ascontiguousarray(
        a["global_weight"].reshape(KC, 128).T).astype(bf)
    wn = np.ascontiguousarray(a["Wn"].T).astype(bf)
    we = np.ascontiguousarray(a["We"].T).astype(bf)

    in_maps = []
    for c in range(N_CORES):
        owned = np.nonzero(heads2 // I2P == c)[0]
        assert len(owned) <= C, f"core {c} owns {len(owned)} > {C} edges"
        # lhs = [ef1_owned | x1_owned]^T, bf16, pre-permuted [128, KC*LW]
        ef1o = np.zeros((C, D), np.float32)
        ef1o[:len(owned)] = a["ef1"][owned]
        lhs_f = np.concatenate(
            [ef1o.T, a["x1"][I2P * c:I2P * (c + 1)].T], axis=1)  # [D, LW]
        lhs = np.ascontiguousarray(
            lhs_f.reshape(KC, 128, LW).transpose(1, 0, 2)
            .reshape(128, KC * LW)).astype(bf)
        # cv[s, i2] = rotated tail + 64*(i2%2) if head matches else 999;
        # rows C..C+5: route mp-diag delta row C+i2 to output row 64*(i2%2)
        cvm = np.full((CD, 6), 999.0, np.float32)
        for s, j2 in enumerate(owned):
            hl = heads2[j2] - I2P * c
            cvm[s, hl] = (tails2[j2] - I2P * c - hl) % N + 64 * (hl % 2)
        for i2 in range(I2P):
            cvm[C + i2, i2] = 64 * (i2 % 2)
        in_maps.append({
            "Wn": wn, "We": we, "gw": np.ascontiguousarray(gw),
            "bnbe": np.ascontiguousarray(bnbe),
            "lhs": lhs, "rhs": rhs, "b1": b1, "cv": cvm,
        })
    return in_maps


def kernel(**inputs) -> np.ndarray:
    global LAST_RESULTS
    nc = _build()
    a = {k: np.ascontiguousarray(np.asarray(v)) for k, v in inputs.items()}
    in_maps = _make_in_maps(a)
    res = run_bass_kernel_spmd(nc, in_maps, core_ids=list(range(N_CORES)))
    LAST_RESULTS = res

    parts = []
    for c in range(N_CORES):
        # device rows are [i2l, k2rot, (i1, k1)] with
        # k2g = (k2rot + i2l + 6c) mod 48; want [i2l, i1, (k2g, k1)]
        o = res.results[c]["out"].reshape(I2P, N, N, N).transpose(0, 2, 1, 3)
        o = np.stack([np.roll(o[i], i + I2P * c, axis=1)
                      for i in range(I2P)])
        parts.append(o.reshape(ROWS, COLS))
    return np.concatenate(parts, axis=0).astype(np.float32)


if __name__ == "__main__":
    _build()
    print("build OK")
